# revision 24
# baseline (speedup 1.0000x reference)
"""Causal self-attention (B=4, T=2048, C=1024, H=16) on 8 trn2 NeuronCores.

Sharding: core c -> (batch b = c//2, head-group g = c%2 of 8 heads).
Each core computes its batch's QKV for its 8 heads, causal attention,
and a partial output projection (its heads' rows of w_out). Host sums
the two partials per batch and adds b_out (+ the v-bias folded through
w_out, which the device never sees).

v2: bf16 end-to-end on the device (inputs converted host-side), one
fused instruction stream per iteration: QKV t-slot matmuls, V tiles,
causal score matmuls, exp on the scalar engine, AV accumulation one
head behind, softmax normalization (ones-matmul broadcast of the
reciprocal denominator), and the output projection woven into the
tail together with the next iteration's prologue (qkT slots 0/4,
head-0 scores, V tiles) computed from prefetched x. The v-bias is
folded through w_out on the host.
"""
import sys
sys.path.insert(0, "/opt/trn_rl_repo")

import numpy as np
import ml_dtypes
import concourse.bass as bass
import concourse.mybir as mybir
import concourse.tile as tile
from concourse import bacc
from concourse.bass_utils import run_bass_kernel_spmd
from concourse.tile import TileContext

F32 = mybir.dt.float32
F32R = mybir.dt.float32r
BF16 = mybir.dt.bfloat16
AF = mybir.ActivationFunctionType

B, T, C = 4, 2048, 1024
H, D = 16, 64
HL = 8            # heads per core
PAIRS = HL // 2   # head pairs (128-partition stacking)
KCH = C // 128    # contraction chunks
NKT = T // 128    # 128-wide key tiles
SCALE = D ** -0.5

_cache = {}


def _build(loop=1, phases=4, unroll=1):
    from contextlib import nullcontext
    nc = bacc.Bacc("TRN2", target_bir_lowering=False, debug=False, num_devices=8)

    xt_d = nc.dram_tensor("xt", [C, T], BF16, kind="ExternalInput")
    wqk_d = nc.dram_tensor("wqk", [C, 1024], BF16, kind="ExternalInput")
    wv_d = nc.dram_tensor("wv", [C, 512], BF16, kind="ExternalInput")
    bqk_d = nc.dram_tensor("bqk", [128, 8], F32, kind="ExternalInput")
    wo_d = nc.dram_tensor("wo", [512, 1024], BF16, kind="ExternalInput")
    y_d = nc.dram_tensor("y", [T, C], F32, kind="ExternalOutput")

    with TileContext(nc) as tc:
        with tc.tile_pool(name="persist", bufs=1) as persist, \
             tc.tile_pool(name="scp", bufs=2, space="PSUM") as sc_pool:
            qkT = persist.tile([128, 8, T], BF16)      # 0-3 q pairs, 4-7 k
            v_aug = persist.tile([128, NKT, HL, D + 1], BF16)
            attout = persist.tile([128, PAIRS, T], BF16)
            bqk_sb = persist.tile([128, 8], F32)
            wqk_sb = persist.tile([128, KCH, 1024], BF16)
            wv_sb = persist.tile([128, KCH, 512], BF16)
            wo_sb = persist.tile([128, PAIRS, 1024], BF16)
            xts = persist.tile([128, KCH, T], BF16)
            den = persist.tile([1, 512], F32R)
            rmap = persist.tile([64, 512], F32)
            ones1 = persist.tile([1, 128], BF16)
            ones1_f = persist.tile([1, 128], F32)

            nc.sync.dma_start(out=bqk_sb, in_=bqk_d[:])
            nc.vector.memset(v_aug[:, :, :, D:D + 1], 1.0)
            nc.gpsimd.memset(attout, 0.0)
            nc.vector.memset(ones1_f, 1.0)
            nc.vector.tensor_copy(ones1, ones1_f)

            with tc.tile_pool(name="atts", bufs=2) as att_pool, \
                 tc.tile_pool(name="accps", bufs=2, space="PSUM") \
                    as acc_ps, \
                 tc.tile_pool(name="avps", bufs=2, space="PSUM") \
                    as av_ps, \
                 tc.tile_pool(name="ysbp", bufs=2) as ypool:
                    at_map = {}

                    def qk_unit(t, gi):
                        ps = acc_ps.tile([128, 512], F32, tag="acc",
                                         name="ps")
                        for k in range(KCH):
                            nc.tensor.matmul(
                                ps, wqk_sb[:, k, 128 * t:128 * (t + 1)],
                                xts[:, k, 512 * gi:512 * (gi + 1)],
                                start=(k == 0), stop=(k == KCH - 1))
                        nc.vector.tensor_scalar_add(
                            qkT[:, t, 512 * gi:512 * (gi + 1)], ps,
                            bqk_sb[:, t:t + 1])

                    def v_unit(tau):
                        ps = acc_ps.tile([128, 512], F32, tag="acc",
                                         name="ps")
                        for k in range(KCH):
                            nc.tensor.matmul(
                                ps, xts[:, k, 128 * tau:128 * (tau + 1)],
                                wv_sb[:, k, :],
                                start=(k == 0), stop=(k == KCH - 1))
                        nc.vector.tensor_copy(
                            v_aug[:, tau, :, 0:D],
                            ps.rearrange("p (h d) -> p h d", h=HL))

                    def score_unit(h, ki):
                        p, r = h // 2, 64 * (h % 2)
                        qlo = 128 * ki
                        at = att_pool.tile([128, T - qlo], BF16,
                                           tag=f"at{ki}", name="at")
                        at_map[(h, ki)] = at
                        lhsT = qkT[r:r + 64, 4 + p, qlo:qlo + 128]
                        qc = qlo
                        while qc < T:
                            qe = min(qc + 1024, T)
                            sc = sc_pool.tile([128, 1024], F32, tag="sc",
                                              name="sc")
                            mc = qc
                            while mc < qe:
                                me = min(mc + 512, qe)
                                nc.tensor.matmul(
                                    sc[:, mc - qc:me - qc], lhsT,
                                    qkT[r:r + 64, p, mc:me],
                                    start=True, stop=True)
                                mc = me
                            nc.scalar.activation(
                                at[:, qc - qlo:qe - qlo], sc[:, 0:qe - qc],
                                AF.Exp, scale=SCALE)
                            if qc == qlo:
                                # zero strictly-upper part of diagonal block
                                nc.gpsimd.affine_select(
                                    out=at[:, 0:128], in_=at[:, 0:128],
                                    compare_op=mybir.AluOpType.is_ge,
                                    fill=0.0, base=0, pattern=[[1, 128]],
                                    channel_multiplier=-1)
                            qc = qe

                    av_tiles = {}

                    def av_mm(h, gi, ki, first, last):
                        if first:
                            av_tiles[(h, gi)] = av_ps.tile(
                                [128, 512], F32, tag="av", name="av")
                        av_t = av_tiles[(h, gi)]
                        g0, qlo = 512 * gi, 128 * ki
                        lo = max(g0, qlo)
                        nc.tensor.matmul(
                            av_t[0:65, lo - g0:512],
                            v_aug[:, ki, h, :],
                            at_map[(h, ki)][:, lo - qlo:512 * (gi + 1) - qlo],
                            start=first, stop=last)

                    def norm_unit(h, gi):
                        p, r = h // 2, 64 * (h % 2)
                        av_t = av_tiles.pop((h, gi))
                        if not norm_on:
                            return
                        nc.vector.tensor_copy(den, av_t[64:65, :])
                        mp = acc_ps.tile([128, 512], F32, tag="acc",
                                         name="mp")
                        nc.tensor.matmul(mp[0:64, :], ones1[:, 0:64], den,
                                         start=True, stop=True)
                        nc.vector.reciprocal_approx_fast(rmap, mp[0:64, :])
                        nc.vector.tensor_mul(
                            attout[r:r + 64, p, 512 * gi:512 * (gi + 1)],
                            av_t[0:64, :], rmap)

                    def av_queue(h):
                        # bursts of 4 accumulation matmuls per weave unit:
                        # denser PE bursts than per-matmul interleaving
                        units = []
                        for gi in range(4):
                            n = 4 * gi + 4
                            for j0 in range(0, n, 4):
                                units.append(("avburst", h, gi, j0,
                                              min(j0 + 4, n), n))
                            units.append(("norm", h, gi))
                        return units

                    def emit(u):
                        if u[0] == "avburst":
                            _, h, gi, j0, j1, n = u
                            for j in range(j0, j1):
                                av_mm(h, gi, j, j == 0, j == n - 1)
                        elif u[0] == "norm":
                            norm_unit(u[1], u[2])
                        elif u[0] == "proj":
                            proj_unit(u[1], u[2])
                        elif u[0] == "qk":
                            qk_unit(u[1], u[2])
                        elif u[0] == "v":
                            v_unit(u[1])

                    def proj_unit(tau, eg):
                        ps = acc_ps.tile([128, 512], F32, tag="acc",
                                         name="ps")
                        for p in range(PAIRS):
                            lhsT = (wqk_sb[:, p, 128 * (tau % 8):
                                           128 * (tau % 8) + 128]
                                    if phases == 8 else
                                    attout[:, p, 128 * tau:128 * (tau + 1)])
                            nc.tensor.matmul(
                                ps, lhsT,
                                wo_sb[:, p, 512 * eg:512 * (eg + 1)],
                                start=(p == 0), stop=(p == PAIRS - 1))
                        if phases in (6, 8):
                            return
                        ysb = ypool.tile([128, 512], F32, tag="ysb",
                                         name="ysb")
                        nc.vector.tensor_copy(ysb, ps)
                        if phases == 5:
                            return
                        nc.sync.dma_start(
                            out=y_d[128 * tau:128 * (tau + 1),
                                    512 * eg:512 * (eg + 1)],
                            in_=ysb)

                    def dma_wqk_x():
                        nc.sync.dma_start(
                            out=wqk_sb,
                            in_=wqk_d.rearrange("(k p) c -> p k c", p=128))
                        nc.sync.dma_start(
                            out=xts,
                            in_=xt_d.rearrange("(k p) t -> p k t", p=128))

                    # ---- pre-loop: loads + iteration-0 prologue (qkT
                    # slots 0/4, head-0 scores, V tiles 0-7) ----
                    dma_wqk_x()
                    nc.sync.dma_start(
                        out=wv_sb,
                        in_=wv_d.rearrange("(k p) c -> p k c", p=128))
                    nc.sync.dma_start(
                        out=wo_sb,
                        in_=wo_d.rearrange("(p c) e -> c p e", c=128))
                    for t in (0, 4):
                        for gi in range(4):
                            qk_unit(t, gi)
                    if phases >= 2:
                        for ki in range(NKT):
                            score_unit(0, ki)
                    for tau in range(8):
                        v_unit(tau)

                    def body():
                        # output projection of the PREVIOUS iteration's
                        # attout (epilogue after the loop produces the
                        # final y), woven into window 1
                        # heads 1-7; between score units: QKV t-slots two
                        # heads ahead of their consumers, V units, and AV
                        # of head h-1
                        fill = {
                            1: ([("proj", tau, eg) for tau in range(NKT)
                                 for eg in range(2)] if phases >= 4 else [])
                               + [("v", tau) for tau in range(8, NKT)]
                               + [("qk", t, gi) for t in (1, 5)
                                  for gi in range(4)],
                            2: [("qk", t, gi) for t in (2, 6)
                                for gi in range(4)],
                            4: [("qk", t, gi) for t in (3, 7)
                                for gi in range(4)],
                        }
                        for h in range(1, HL if phases >= 2 else 1):
                            weave = fill.get(h, [])
                            if phases >= 3:
                                weave = weave + av_queue(h - 1)
                            done = 0
                            for ki in range(NKT):
                                score_unit(h, ki)
                                tgt = len(weave) * (ki + 1) // NKT
                                while done < tgt:
                                    emit(weave[done])
                                    done += 1
                            if h == 1:
                                # refresh wv for the tail's V units (its
                                # last readers were this window's V units)
                                # and wo for the next proj sweep
                                nc.sync.dma_start(
                                    out=wv_sb,
                                    in_=wv_d.rearrange("(k p) c -> p k c",
                                                       p=128))
                                nc.sync.dma_start(
                                    out=wo_sb,
                                    in_=wo_d.rearrange("(p c) e -> c p e",
                                                       c=128))
                            if h == 4:
                                # next iteration's x / wqk: the last
                                # readers (qk slots 3,7 and V units) are
                                # done; transfer hides under heads 5-7
                                dma_wqk_x()

                        # tail: AV of head 7 + output projection, woven
                        # with the next iteration's prologue (qkT slots
                        # 0/4, head-0 scores, V tiles 0-7 computed from
                        # the prefetched x)
                        avq = av_queue(HL - 1) if phases >= 3 else []
                        pro = [("qk", t, gi) for t in (0, 4)
                               for gi in range(4)]
                        if phases >= 2:
                            pro += [("sc0", ki) for ki in range(NKT)]
                        done = 0
                        pdone = 0
                        for gi in range(4):
                            while done < len(avq):
                                emit(avq[done])
                                done += 1
                                if avq[done - 1][0] == "norm":
                                    break
                            if gi == 3:
                                # v_aug free only once AV of head 7 done
                                pro += [("v", tau) for tau in range(8)]
                            ptgt = len(pro) * (gi + 1) // 4
                            for tau in range(4 * gi, 4 * gi + 4):
                                if proj_on:
                                    for eg in range(2):
                                        proj_unit(tau, eg)
                                while pdone < ptgt * (tau % 4 + 1) // 4:
                                    u = pro[pdone]
                                    if u[0] == "sc0":
                                        score_unit(0, u[1])
                                    else:
                                        emit(u)
                                    pdone += 1
                            while pdone < ptgt:
                                u = pro[pdone]
                                if u[0] == "sc0":
                                    score_unit(0, u[1])
                                else:
                                    emit(u)
                                pdone += 1

                    if unroll > 1:
                        for _ in range(unroll):
                            body()
                    else:
                        loop_cm = (tc.For_i(0, loop, 1) if loop > 1
                                   else nullcontext())
                        with loop_cm:
                            body()
                    # epilogue: project the final iteration's attout
                    if phases >= 4:
                        for tau in range(NKT):
                            for eg in range(2):
                                proj_unit(tau, eg)

    nc.compile()
    return nc


def _prep_inputs(x, w_qkv, b_qkv, w_out, b_out):
    BF = ml_dtypes.bfloat16
    x = np.asarray(x, np.float32)
    w_qkv = np.asarray(w_qkv, np.float32)
    b_qkv = np.asarray(b_qkv, np.float32)
    w_out = np.asarray(w_out, np.float32)
    in_maps = []
    for c in range(8):
        b, g = c // 2, c % 2
        xt = np.ascontiguousarray(x[b].T).astype(BF)
        wqk = np.concatenate(
            [w_qkv[:, 512 * g:512 * g + 512],
             w_qkv[:, C + 512 * g:C + 512 * g + 512]], axis=1).astype(BF)
        bqk = np.concatenate(
            [b_qkv[512 * g:512 * g + 512],
             b_qkv[C + 512 * g:C + 512 * g + 512]]).reshape(8, 128).T
        wv = w_qkv[:, 2 * C + 512 * g:2 * C + 512 * g + 512].astype(BF)
        wo = w_out[512 * g:512 * g + 512, :].astype(BF)
        in_maps.append({
            "xt": np.ascontiguousarray(xt),
            "wqk": np.ascontiguousarray(wqk),
            "bqk": np.ascontiguousarray(bqk),
            "wv": np.ascontiguousarray(wv),
            "wo": np.ascontiguousarray(wo),
        })
    return in_maps


def kernel(x, w_qkv, b_qkv, w_out, b_out):
    if "nc" not in _cache:
        _cache["nc"] = _build()
    nc = _cache["nc"]
    in_maps = _prep_inputs(x, w_qkv, b_qkv, w_out, b_out)
    res = run_bass_kernel_spmd(nc, in_maps, list(range(8)))
    b_qkv = np.asarray(b_qkv, np.float64)
    w_out64 = np.asarray(w_out, np.float64)
    # v-bias folded through w_out on the host (exact), plus b_out
    yb = (np.asarray(b_out, np.float64)
          + b_qkv[2 * C:3 * C] @ w_out64).astype(np.float32)
    out = np.empty((B, T, C), np.float32)
    for b in range(B):
        out[b] = res.results[2 * b]["y"] + res.results[2 * b + 1]["y"] + yb
    return out


def bench(x, w_qkv, b_qkv, w_out, b_out, iters=16, reps=3, loop=None, phases=4):
    """Time the NEFF on hardware. The kernel body is wrapped in a For_i
    hardware loop of `iters` iterations (one dispatch); subtracting the
    1-iteration dispatch time cancels network/dispatch overhead.
    Returns per-execution seconds."""
    import time
    import jax
    import jax.numpy as jnp
    from jax.sharding import Mesh, PartitionSpec
    from jax.experimental.shard_map import shard_map
    from concourse import bass2jax
    from concourse.bass2jax import (
        _bass_exec_p, install_neuronx_cc_hook, partition_id_tensor)

    if (loop is not None and loop > 1) or phases != 4:
        nc = _build(loop=loop or 1, phases=phases)
    else:
        nc = _cache.setdefault("nc", _build())
    install_neuronx_cc_hook()
    in_maps = _prep_inputs(x, w_qkv, b_qkv, w_out, b_out)

    partition_name = (nc.partition_id_tensor.name
                      if nc.partition_id_tensor else None)
    in_names, out_names, out_avals, zero_outs = [], [], [], []
    for alloc in nc.m.functions[0].allocations:
        if not isinstance(alloc, mybir.MemoryLocationSet):
            continue
        name = alloc.memorylocations[0].name
        if alloc.kind == "ExternalInput":
            if name != partition_name:
                in_names.append(name)
        elif alloc.kind == "ExternalOutput":
            out_names.append(name)
            shape = tuple(alloc.tensor_shape)
            dtype = mybir.dt.np(alloc.dtype)
            out_avals.append(jax.core.ShapedArray(shape, dtype))
            zero_outs.append(np.zeros(shape, dtype))
    n_params = len(in_names)
    all_names = in_names + out_names
    if partition_name is not None:
        all_names.append(partition_name)
    chain_idx = in_names.index("bqk")

    def body_n(n):
        def _body(*args):
            ins = list(args)
            outs = None
            for _ in range(n):
                cur = list(ins)
                if outs is not None:
                    y = outs[0]
                    cur[chain_idx] = cur[chain_idx] + 0.0 * y[:128, :8]
                if partition_name is not None:
                    cur.append(partition_id_tensor())
                outs = _bass_exec_p.bind(
                    *cur,
                    out_avals=tuple(out_avals),
                    in_names=tuple(all_names),
                    out_names=tuple(out_names),
                    lowering_input_output_aliases=(),
                    sim_require_finite=True,
                    sim_require_nnan=True,
                    nc=nc,
                )
            return tuple(outs)
        return _body

    devices = jax.devices()[:8]
    mesh = Mesh(np.asarray(devices), ("core",))
    in_specs = (PartitionSpec("core"),) * (n_params + len(out_names))
    out_specs = (PartitionSpec("core"),) * len(out_names)

    per_core = [[np.asarray(m[name]) for name in in_names] for m in in_maps]
    concat_in = [np.concatenate([per_core[c][i] for c in range(8)], axis=0)
                 for i in range(n_params)]
    concat_zero = [np.zeros((8 * z.shape[0], *z.shape[1:]), z.dtype)
                   for z in zero_outs]
    ins_dev = [jax.device_put(a) for a in concat_in]
    donate = tuple(range(n_params, n_params + len(zero_outs)))

    f = jax.jit(shard_map(body_n(1), mesh=mesh, in_specs=in_specs,
                          out_specs=out_specs, check_rep=False),
                donate_argnums=donate, keep_unused=True)

    def fresh_zeros(n):
        return [[jax.device_put(z) for z in concat_zero] for _ in range(n)]

    z0 = fresh_zeros(1)[0]
    jax.block_until_ready(f(*ins_dev, *z0))  # compile + warm

    def timed():
        best = float("inf")
        for _ in range(reps):
            zs = fresh_zeros(1)[0]
            jax.block_until_ready(zs)
            t0 = time.perf_counter()
            r = f(*ins_dev, *zs)
            jax.block_until_ready(r)
            best = min(best, time.perf_counter() - t0)
        return best

    return timed()
